# revision 2
# baseline (speedup 1.0000x reference)
"""Trainium2 Bass kernel for nn_ConvFCLIFNet.

Pipeline: x_seq (T=64, B=512, 1, 28, 28) -> conv2x2(valid) -> FC(729) -> LIF
scan over T -> spike sequence (T, B, 729) in {0.0, 1.0}.

Strategy
--------
- conv + FC + bias + 1/tau fold into ONE matmul: y*0.5 = x_aug @ W_aug where
  x_aug = [x_pixels(784), 1.0] and W_aug[p, o] = 0.5 * (fc_w @ C)^T (C = conv
  scatter), bias row at p=784.
- Data-parallel over 8 NeuronCores: 64 samples each.
- Matmul: W chunks stationary [128 pixels, 128 features] (f32r — full PE rate,
  ~12.5 effective mantissa bits), x^T moving [128 pixels, G*64 samples].
  PSUM output [128 features, NJ=6 chunks, G*64] -> partition dim is FEATURES,
  so the LIF state q [128, 6, 64] stays on fixed partitions all 64 steps.
- LIF scan: ONE custom DVE op per timestep:
      u = (q_prev == SENT) ? 0 : q_prev;  w = z + u
      q = (w >= 1) ? SENT : 0.5 * w
  Spike decode on ScalarE: s = Relu(q + (1 - SENT)) -> exactly 1.0 iff spiked.
- Host does only layout staging (shard, pixel-major transpose, weight fold)
  plus the final gather/decode.
"""
import numpy as np

import concourse.bacc as bacc
import concourse.mybir as mybir
import concourse.tile as tile
from concourse.bass_utils import run_bass_kernel_spmd

# ---------------------------------------------------------------- constants
T, B, H, W = 64, 512, 28, 28
NPIX = H * W            # 784
NF = 729                # fc features
NCORES = 8
BS = B // NCORES        # 64 samples per core
import os
G = int(os.environ.get("LIF_G", "8"))   # timesteps per matmul group
NG = T // G
NJ = 6                  # feature chunks of 128 (768 padded)
KT = 7                  # contraction k-tiles: 6 x 128 + 17 (784 pixels + bias)
KTAIL = NPIX + 1 - 6 * 128   # 17
NS = G * BS             # moving free size = 256
SENT = float(2 ** 20)

_CACHE = {}

# ------------------------------------------------------------ custom DVE op

def _register_lif_op():
    from concourse.dve_spec import Spec, Src0, Src1, C0, C1, Zero, One, select, eq, lower
    from concourse.dve_uop import DveOpSpec
    from concourse import dve_ops

    name = "LIF_STEP_ANT"
    for op in dve_ops.OPS:
        if op.name == name:
            return op

    def _ref(in0, in1, s0, s1, imm2=None):
        u = np.where(in1 == s0, 0.0, in1).astype(np.float32)
        w = (in0 + u).astype(np.float32)
        return np.where(w >= 1.0, np.float32(s0), (w * np.float32(s1)).astype(np.float32))

    _u = select(eq(Src1, C0), Zero, Src1)
    _w = Src0 + _u
    spec = Spec(body=select(_w >= One, C0, _w * C1), reference=_ref)

    row = dve_ops._CUSTOM_DVE_ROW_BASE + len(dve_ops.OPS)
    assert row < 0x20
    dve_ops._SUB_OPCODE_FOR_NAME[name] = row
    shas = {}
    for ver in ("v3", "v4"):
        s = DveOpSpec(name=name, opcode=row, uops=lower(spec, ver=ver), rd1_en=True)
        shas[ver] = s.sha(ver)
    op = dve_ops.DveOp(name, spec, subdim=False, uops_sha=shas)
    dve_ops.OPS.append(op)
    dve_ops.CUSTOM_DVE_SPECS[name] = spec
    return op

# ------------------------------------------------------------- device build

def _build(reps: int = 1):
    lif = _register_lif_op()
    nc = bacc.Bacc(None, target_bir_lowering=False, debug=False)
    f32, f32r = mybir.dt.float32, mybir.dt.float32r
    with tile.TileContext(nc) as tc:
        with tc.tile_pool(name="dram", bufs=1, space="DRAM") as dram, \
             tc.tile_pool(name="consts", bufs=1) as consts, \
             tc.tile_pool(name="xpool", bufs=3) as xpool, \
             tc.tile_pool(name="qpool", bufs=2) as qpool, \
             tc.tile_pool(name="spool", bufs=4) as spool, \
             tc.tile_pool(name="pspool", bufs=(2 if G <= 4 else 1), space="PSUM") as pspool:
            x_in = dram.tile([NG, NPIX + 1, NS], f32r, kind="ExternalInput",
                             name="x_in", uniquify=False)
            w_in = dram.tile([KT, 128, NJ, 128], f32r, kind="ExternalInput",
                             name="w_in", uniquify=False)
            out = dram.tile([T, 128, NJ, BS], f32, kind="ExternalOutput",
                            name="out", uniquify=False)

            wsb = consts.tile([128, KT, NJ, 128], f32r)
            nc.sync.dma_start(out=wsb[:, :, :, :], in_=w_in.rearrange("k p j m -> p k j m"))
            bias_t = consts.tile([128, 1], f32)
            nc.vector.memset(bias_t[:, :], float(1.0 - SENT))

            q = qpool.tile([128, NJ, BS], f32, name="q", tag="q")
            nc.vector.memset(q[:, :, :], 0.0)

            for rep in range(reps):
                if rep > 0:
                    # timing-only extra passes reuse state; reset q
                    q = qpool.tile([128, NJ, BS], f32, name="q", tag="q")
                    nc.vector.memset(q[:, :, :], 0.0)
                for g in range(NG):
                    x_sb = xpool.tile([128, KT, NS], f32r, name="x_sb", tag="x")
                    nc.sync.dma_start(
                        out=x_sb[:, 0:6, :],
                        in_=x_in[g, 0:768, :].rearrange("(k p) n -> p k n", p=128),
                    )
                    nc.sync.dma_start(
                        out=x_sb[0:KTAIL, 6, :],
                        in_=x_in[g, 768:NPIX + 1, :],
                    )
                    ps = pspool.tile([128, NJ, NS], f32, name="ps", tag="ps")
                    for j in range(NJ):
                        for kt in range(6):
                            nc.tensor.matmul(
                                ps[:, j, :],
                                lhsT=wsb[:, kt, j, :],
                                rhs=x_sb[:, kt, :],
                                start=(kt == 0), stop=False,
                            )
                        nc.tensor.matmul(
                            ps[:, j, :],
                            lhsT=wsb[0:KTAIL, 6, j, :],
                            rhs=x_sb[0:KTAIL, 6, :],
                            start=False, stop=True,
                        )
                    for tl in range(G):
                        t = g * G + tl
                        q2 = qpool.tile([128, NJ, BS], f32, name="q", tag="q")
                        nc.vector._custom_dve(
                            lif,
                            out=q2[:, :, :],
                            in0=ps[:, :, tl * BS:(tl + 1) * BS],
                            in1=q[:, :, :],
                            s0=SENT, s1=0.5,
                        )
                        s_sb = spool.tile([128, NJ, BS], f32, name="s_sb", tag="s")
                        nc.scalar.activation(
                            s_sb[:, :, :], q2[:, :, :],
                            mybir.ActivationFunctionType.Relu,
                            bias=bias_t[:, :], scale=1.0,
                        )
                        nc.sync.dma_start(out=out[t], in_=s_sb[:, :, :])
                        q = q2
    nc.compile()
    return nc

# --------------------------------------------------------------- host side

def _prep_weights(conv_w, fc_w, fc_b):
    """W_aug [KT,128,NJ,128]: rows = pixels (784) + bias row (784) + pad,
    cols = 768 features (729 + pad); scaled by 0.5 (tau fold)."""
    cw = conv_w.reshape(2, 2).astype(np.float32)
    fcw = fc_w.astype(np.float32).reshape(NF, 27, 27)
    tmp = np.zeros((NF, H, W), np.float32)
    for dr in range(2):
        for dc in range(2):
            tmp[:, dr:dr + 27, dc:dc + 27] += cw[dr, dc] * fcw
    w_eff = tmp.reshape(NF, NPIX)                     # [729, 784]
    w_aug = np.zeros((KT * 128, NJ * 128), np.float32)
    w_aug[:NPIX, :NF] = 0.5 * w_eff.T
    w_aug[NPIX, :NF] = 0.5 * fc_b.astype(np.float32)
    return np.ascontiguousarray(
        w_aug.reshape(KT, 128, NJ, 128))

def _prep_x(x_seq):
    """Per-core pixel-major inputs [NCORES][NG, 785, G*64]."""
    xs = np.ascontiguousarray(x_seq.reshape(T, NCORES, BS, NPIX))
    # -> [core, group, pixel, (tl, sample)]
    xt = xs.transpose(1, 0, 3, 2).reshape(NCORES, NG, G, NPIX, BS)
    xt = xt.transpose(0, 1, 3, 2, 4).reshape(NCORES, NG, NPIX, NS)
    xp = np.empty((NCORES, NG, NPIX + 1, NS), np.float32)
    xp[:, :, :NPIX, :] = xt
    xp[:, :, NPIX, :] = 1.0
    return xp

def kernel(x_seq, conv_w, fc_w, fc_b):
    if "nc" not in _CACHE:
        _CACHE["nc"] = _build(reps=1)
    nc = _CACHE["nc"]
    w_aug = _prep_weights(conv_w, fc_w, fc_b)
    xp = _prep_x(np.asarray(x_seq, dtype=np.float32))
    in_maps = [{"x_in": np.ascontiguousarray(xp[c]), "w_in": w_aug}
               for c in range(NCORES)]
    res = run_bass_kernel_spmd(nc, in_maps, core_ids=list(range(NCORES)))
    _CACHE["last_res"] = res
    full = np.empty((T, B, NF), np.float32)
    for c in range(NCORES):
        o = res.results[c]["out"]                     # [T, 128, NJ, BS]
        # feature f = j*128 + p ; sample s
        full[:, c * BS:(c + 1) * BS, :] = (
            o.transpose(0, 3, 2, 1).reshape(T, BS, NJ * 128)[:, :, :NF])
    return full

